# revision 9
# baseline (speedup 1.0000x reference)
"""GAT layer kernel for Trainium2, sharded across 8 NeuronCores.

Math: since adj is 0/1 and the attention logit e_i is constant across row i,
the masked softmax collapses to attention[i,j] = adj[i,j] / rowdeg(i), so

    out = elu((adj @ h) / d),   h = x @ W,   d = adj @ ones

Per-core strategy (core c owns destination rows R_c = [c*1536, (c+1)*1536)):
  - HBM traffic is the roofline (~358 GB/s per core), so the host packs the
    inputs into the smallest exact dtypes: adj (0/1 int32) becomes fp8-e4m3
    (values 0.0/1.0 are exact, 4x fewer bytes) and x/W become bf16.
  - host passes adjT8[p, kb*1536+m] = adj[c*1536+m, kb*128+p]: the shard
    transposed and k-blocked so each 8-block DMA chunk reads 12 KB
    contiguous per partition.
  - all bulk loads share the gpsimd (SWDGE) queue in priority order (x
    chunks first, then adj chunks) so the h = x@W phase starts ~10 us in
    and overlaps the adj stream end to end.
  - device computes full h once (x replicated), augmented with a ones column
    -> h_aug [12288, 65] bf16; the PE accumulates
    s_aug^T[65, 1536] += h_aug[kb].T (bf16 stationary) @ adjT8[kb] (fp8
    moving) -- the PE allows mixed operand dtypes (both upconvert to fp22).
  - the last chunk runs mt-major so each 512-column PSUM region finishes
    early and its epilogue (PE transpose back to row-major, ACT ops with
    the 1/deg division fused in as a per-partition scale, ELU) overlaps the
    remaining matmuls; output staged and stored in 3 slabs.
The adj traffic (18.9 MB fp8 per core) is the memory roofline.
"""

import numpy as np

_N = 12288
_P = 128
_NCORES = 8
_ROWS = _N // _NCORES          # 1536 destination rows per core
_KB = _N // _P                 # 96 k-blocks
_INF = 256
_OUTF = 64
_HA = _OUTF + 1                # h augmented with ones column
_MT = _ROWS // 512             # 3 moving-operand tiles per k-block
_CH = 8                        # k-blocks per DMA chunk (1.57 MB each)
_NCH = _KB // _CH              # 12 chunks
_XCH = 8                       # x column-chunks per 128-row half
_HG = 8                        # h blocks per PSUM group
_TPG = (_ROWS // _P) // _MT    # epilogue row-blocks per mt region (4)

_cached_nc = None
last_results = None            # BassKernelResults of the most recent run


def _build_nc():
    from contextlib import ExitStack

    import concourse.bacc as bacc
    import concourse.mybir as mybir
    import concourse.tile as tile
    from concourse.masks import make_identity

    f32 = mybir.dt.float32
    bf16 = mybir.dt.bfloat16
    f8 = mybir.dt.float8e4
    ACT = mybir.ActivationFunctionType

    nc = bacc.Bacc("TRN2", target_bir_lowering=False, debug=False)
    adjT8 = nc.dram_tensor("adjT8", [_P, _KB * _ROWS], f8, kind="ExternalInput")
    xT = nc.dram_tensor("xT", [_INF, _N], bf16, kind="ExternalInput")
    W = nc.dram_tensor("W", [_INF, _OUTF], bf16, kind="ExternalInput")
    # raw staging layout [partition, t*64+f]; host reassembles rows as
    # out[t*128+p, f] = out_raw[p, t*64+f]. Keeps the store at 1KB/partition
    # contiguous chunks (a [1536, 64] row-major store would be 256B chunks,
    # under the 512B line-rate minimum -> RMW-slow).
    out = nc.dram_tensor("out", [_P, (_ROWS // _P) * _OUTF], f32,
                         kind="ExternalOutput")

    with ExitStack() as ctx:
        tc = ctx.enter_context(tile.TileContext(nc))
        cpool = ctx.enter_context(tc.tile_pool(name="cpool", bufs=1))
        xpool = ctx.enter_context(tc.tile_pool(name="xpool", bufs=1))
        hpool = ctx.enter_context(tc.tile_pool(name="hpool", bufs=1))
        apool = ctx.enter_context(tc.tile_pool(name="apool", bufs=9))
        epool = ctx.enter_context(tc.tile_pool(name="epool", bufs=6))
        ps_main = ctx.enter_context(tc.tile_pool(name="ps_main", bufs=1, space="PSUM"))
        ps_h = ctx.enter_context(tc.tile_pool(name="ps_h", bufs=2, space="PSUM"))
        ps_t = ctx.enter_context(tc.tile_pool(name="ps_t", bufs=3, space="PSUM"))

        ident = cpool.tile([_P, _P], f32, name="ident", tag="ident")
        make_identity(nc, ident[:])

        # small loads on the scalar HWDGE ring; bulk loads go on the gpsimd
        # SWDGE ring in priority order (x first, adj behind it)
        w_sb = cpool.tile([_P, 2 * _OUTF], bf16, name="w_sb", tag="w_sb")
        nc.scalar.dma_start(w_sb[:, 0:_OUTF], W[0:_P, :])
        nc.scalar.dma_start(w_sb[:, _OUTF:], W[_P:, :])

        xw = _N // _XCH
        xts = []
        for j in range(_XCH):
            xt0 = xpool.tile([_P, xw], bf16, name=f"xt0_{j}", tag=f"xt0_{j}")
            nc.gpsimd.dma_start(xt0[:], xT[0:_P, j * xw:(j + 1) * xw])
            xt1 = xpool.tile([_P, xw], bf16, name=f"xt1_{j}", tag=f"xt1_{j}")
            nc.gpsimd.dma_start(xt1[:], xT[_P:, j * xw:(j + 1) * xw])
            xts.append((xt0, xt1))

        ats = []
        for ch in range(_NCH):
            at = apool.tile([_P, _CH * _ROWS], f8, name="at", tag="at")
            nc.gpsimd.dma_start(at[:], adjT8[:, ch * _CH * _ROWS:(ch + 1) * _CH * _ROWS])
            ats.append(at)

        # h_aug blocks: [p, kb, f]; col 64 of each block is the ones column
        # (strided memset once, never rewritten)
        h_aug = hpool.tile([_P, _KB, _HA], bf16, name="h_aug", tag="h_aug")
        nc.vector.memset(h_aug[:, :, _OUTF:_HA], 1.0)

        # h-phase in groups of 8 blocks per PSUM bank; one grouped strided
        # copy per bank, alternating scalar/vector so the two engines halve
        # the copy wall
        nblk = _P // _P  # 1 block per 128 cols of an x chunk pair
        for g in range(_KB // _HG):
            ph = ps_h.tile([_P, _HG, _OUTF], f32, name="ph", tag="ph")
            for j in range(_HG):
                ib = g * _HG + j
                xc, lo = divmod(ib * _P, xw)
                xt0, xt1 = xts[xc]
                nc.tensor.matmul(ph[:, j, :], lhsT=xt0[:, lo:lo + _P],
                                 rhs=w_sb[:, 0:_OUTF], start=True, stop=False)
                nc.tensor.matmul(ph[:, j, :], lhsT=xt1[:, lo:lo + _P],
                                 rhs=w_sb[:, _OUTF:], start=False, stop=True)
            dst = h_aug[:, g * _HG:(g + 1) * _HG, 0:_OUTF]
            if g % 2 == 0:
                nc.scalar.activation(dst, ph[:], ACT.Copy)
            else:
                nc.vector.tensor_copy(dst, ph[:])

        # main accumulation: s_aug^T[f, m] += sum_kb h_aug[kb].T @ adjT8[kb]
        # one PSUM tile per 512-column region so each region's accumulation
        # group closes independently and its epilogue overlaps the rest
        ps_mt = [ps_main.tile([_HA, 512], f32, name=f"ps{mt}", tag=f"ps{mt}")
                 for mt in range(_MT)]

        def mm(kb, mt, at, b):
            nc.tensor.matmul(
                ps_mt[mt][:, :],
                lhsT=h_aug[:, kb, :],
                rhs=at[:, b * _ROWS + mt * 512: b * _ROWS + (mt + 1) * 512],
                start=(kb == 0), stop=(kb == _KB - 1),
                perf_mode=mybir.MatmulPerfMode.DoublePixel,
            )

        for ch in range(_NCH - 1):
            for b in range(_CH):
                for mt in range(_MT):
                    mm(ch * _CH + b, mt, ats[ch], b)

        # last chunk mt-major: each 512-col PSUM region stops early so its
        # epilogue overlaps the remaining matmuls
        out_stage = hpool.tile([_P, (_ROWS // _P) * _OUTF], f32,
                               name="out_stage", tag="out_stage")
        last = ats[_NCH - 1]
        for mt in range(_MT):
            for b in range(_CH):
                mm((_NCH - 1) * _CH + b, mt, last, b)
            for t in range(mt * _TPG, (mt + 1) * _TPG):
                src = ps_mt[mt][:, (t % _TPG) * _P:(t % _TPG + 1) * _P]
                sT = epool.tile([_HA, _P], f32, name="sT", tag="sT")
                if t % 2 == 0:
                    nc.scalar.activation(sT[:], src, ACT.Copy)
                else:
                    nc.vector.tensor_copy(sT[:], src)
                tp = ps_t.tile([_P, _HA], f32, name="tp", tag="tp")
                nc.tensor.transpose(tp[:], sT[:], ident[0:_HA, 0:_HA])
                rec = epool.tile([_P, 1], f32, name="rec", tag="rec")
                nc.vector.reciprocal(rec[:], tp[:, _OUTF:_HA])
                # elu(z) = relu(z) - relu(1 - exp(z)), z = s/deg; the 1/deg
                # division rides along as the ACT per-partition scale
                z = epool.tile([_P, _OUTF], f32, name="z", tag="z")
                nc.scalar.activation(z[:], tp[:, 0:_OUTF], ACT.Relu, scale=rec[:])
                ex = epool.tile([_P, _OUTF], f32, name="ex", tag="ex")
                nc.scalar.activation(ex[:], tp[:, 0:_OUTF], ACT.Exp, scale=rec[:])
                q = epool.tile([_P, _OUTF], f32, name="q", tag="q")
                nc.scalar.activation(q[:], ex[:], ACT.Relu, bias=1.0, scale=-1.0)
                ob = out_stage[:, t * _OUTF:(t + 1) * _OUTF]
                nc.vector.tensor_sub(ob, z[:], q[:])
            s0 = mt * _TPG * _OUTF
            s1 = (mt + 1) * _TPG * _OUTF
            nc.scalar.dma_start(out[:, s0:s1], out_stage[:, s0:s1])

    nc.compile()
    return nc


def _spot_check(out, adj, x, W):
    """Validate a few output rows on host (guards against rare HW transients;
    ~4x the bf16 noise floor). Returns max relative error over the sample."""
    rows = np.arange(_NCORES * 16) * (_N // (_NCORES * 16)) + 7
    h = x.astype(np.float32) @ W.astype(np.float32)
    asel = adj[rows].astype(np.float32)
    s = (asel @ h) / asel.sum(axis=1, keepdims=True)
    want = np.where(s > 0, s, np.expm1(s))
    return np.abs(out[rows] - want).max() / max(np.abs(want).max(), 1e-6)


def kernel(adj, x, W, a=None):
    global _cached_nc, last_results
    from concurrent.futures import ThreadPoolExecutor

    import ml_dtypes
    from concourse.bass_utils import run_bass_kernel_spmd

    adj = np.ascontiguousarray(adj)
    adj8 = adj.astype(ml_dtypes.float8_e4m3)     # 0/1 are exact in fp8
    xT = np.ascontiguousarray(np.asarray(x, dtype=np.float32).T
                              .astype(ml_dtypes.bfloat16))
    Wb = np.asarray(W, dtype=np.float32).astype(ml_dtypes.bfloat16)

    def shard(c):
        # [p, kb*1536+m] = adj8[c*1536+m, kb*128+p]
        s = adj8[c * _ROWS:(c + 1) * _ROWS]      # [1536, 12288]
        return np.ascontiguousarray(
            s.reshape(_ROWS, _KB, _P).transpose(2, 1, 0)
        ).reshape(_P, _KB * _ROWS)

    with ThreadPoolExecutor(_NCORES) as ex:
        shards = list(ex.map(shard, range(_NCORES)))

    if _cached_nc is None:
        _cached_nc = _build_nc()

    in_maps = [{"adjT8": shards[c], "xT": xT, "W": Wb} for c in range(_NCORES)]
    out = None
    for _attempt in range(3):
        try:
            last_results = run_bass_kernel_spmd(
                _cached_nc, in_maps, core_ids=list(range(_NCORES))
            )
        except ModuleNotFoundError:
            # BASS_TRACE set but this image lacks the axon NTFF hook module;
            # rerun with tracing forced off
            import os

            os.environ["BASS_NEVER_TRACE"] = "1"
            last_results = run_bass_kernel_spmd(
                _cached_nc, in_maps, core_ids=list(range(_NCORES))
            )
        out = np.concatenate(
            [
                last_results.results[c]["out"]
                .reshape(_P, _ROWS // _P, _OUTF)
                .transpose(1, 0, 2)
                .reshape(_ROWS, _OUTF)
                for c in range(_NCORES)
            ],
            axis=0,
        ).astype(np.float32)
        if _spot_check(out, adj, x, W) < 1.5e-2:
            break
    return out


# revision 11
# speedup vs baseline: 1.0815x; 1.0815x over previous
"""GAT layer kernel for Trainium2, sharded across 8 NeuronCores.

Math: since adj is 0/1 and the attention logit e_i is constant across row i,
the masked softmax collapses to attention[i,j] = adj[i,j] / rowdeg(i), so

    out = elu((adj @ h) / d),   h = x @ W,   d = adj @ ones

Per-core strategy (core c owns destination rows R_c = [c*1536, (c+1)*1536)):
  - HBM traffic is the roofline (~358 GB/s per core), so the host packs the
    inputs into the smallest exact dtypes: adj (0/1 int32) becomes fp8-e4m3
    (values 0.0/1.0 are exact, 4x fewer bytes) and x/W become bf16.
  - host passes adjT8[p, kb*1536+m] = adj[c*1536+m, kb*128+p]: the shard
    transposed and k-blocked so each 8-block DMA chunk reads 12 KB
    contiguous per partition.
  - all bulk loads share the gpsimd (SWDGE) queue in priority order (x
    chunks first, then adj chunks) so the h = x@W phase starts ~10 us in
    and overlaps the adj stream end to end.
  - device computes full h once (x replicated), augmented with a ones column
    -> h_aug [12288, 65] bf16; the PE accumulates
    s_aug^T[65, 1536] += h_aug[kb].T (bf16 stationary) @ adjT8[kb] (fp8
    moving) -- the PE allows mixed operand dtypes (both upconvert to fp22).
  - the last chunk runs mt-major so each 512-column PSUM region finishes
    early and its epilogue (PE transpose back to row-major, ACT ops with
    the 1/deg division fused in as a per-partition scale, ELU) overlaps the
    remaining matmuls; output staged and stored in 3 slabs.
The adj traffic (18.9 MB fp8 per core) is the memory roofline.
"""

import numpy as np

_N = 12288
_P = 128
_NCORES = 8
_ROWS = _N // _NCORES          # 1536 destination rows per core
_KB = _N // _P                 # 96 k-blocks
_INF = 256
_OUTF = 64
_HA = _OUTF + 1                # h augmented with ones column
_MT = _ROWS // 512             # 3 moving-operand tiles per k-block
_CH = 8                        # k-blocks per DMA chunk (1.57 MB each)
_NCH = _KB // _CH              # 12 chunks
_XCH = 12                      # x column-chunks per 128-row half (1 per group)
_HG = 8                        # h blocks per PSUM group
_TPG = (_ROWS // _P) // _MT    # epilogue row-blocks per mt region (4)

_cached_nc = None
last_results = None            # BassKernelResults of the most recent run


def _build_nc():
    from contextlib import ExitStack

    import concourse.bacc as bacc
    import concourse.mybir as mybir
    import concourse.tile as tile
    from concourse.masks import make_identity

    f32 = mybir.dt.float32
    bf16 = mybir.dt.bfloat16
    f8 = mybir.dt.float8e4
    ACT = mybir.ActivationFunctionType

    nc = bacc.Bacc("TRN2", target_bir_lowering=False, debug=False)
    adjT8 = nc.dram_tensor("adjT8", [_P, _KB * _ROWS], f8, kind="ExternalInput")
    xT = nc.dram_tensor("xT", [_INF, _N], bf16, kind="ExternalInput")
    W = nc.dram_tensor("W", [_INF, _OUTF], bf16, kind="ExternalInput")
    # raw staging layout [partition, t*64+f]; host reassembles rows as
    # out[t*128+p, f] = out_raw[p, t*64+f]. Keeps the store at 1KB/partition
    # contiguous chunks (a [1536, 64] row-major store would be 256B chunks,
    # under the 512B line-rate minimum -> RMW-slow).
    out = nc.dram_tensor("out", [_P, (_ROWS // _P) * _OUTF], f32,
                         kind="ExternalOutput")

    with ExitStack() as ctx:
        tc = ctx.enter_context(tile.TileContext(nc))
        cpool = ctx.enter_context(tc.tile_pool(name="cpool", bufs=1))
        xpool = ctx.enter_context(tc.tile_pool(name="xpool", bufs=1))
        hpool = ctx.enter_context(tc.tile_pool(name="hpool", bufs=1))
        apool = ctx.enter_context(tc.tile_pool(name="apool", bufs=9))
        epool = ctx.enter_context(tc.tile_pool(name="epool", bufs=6))
        ps_main = ctx.enter_context(tc.tile_pool(name="ps_main", bufs=1, space="PSUM"))
        ps_h = ctx.enter_context(tc.tile_pool(name="ps_h", bufs=2, space="PSUM"))
        ps_t = ctx.enter_context(tc.tile_pool(name="ps_t", bufs=3, space="PSUM"))

        ident = cpool.tile([_P, _P], f32, name="ident", tag="ident")
        make_identity(nc, ident[:])

        # small loads on the scalar HWDGE ring; bulk loads go on the gpsimd
        # SWDGE ring in priority order (x first, adj behind it)
        w_sb = cpool.tile([_P, 2 * _OUTF], bf16, name="w_sb", tag="w_sb")
        nc.scalar.dma_start(w_sb[:, 0:_OUTF], W[0:_P, :])
        nc.scalar.dma_start(w_sb[:, _OUTF:], W[_P:, :])

        # DMA issue order on the single SWDGE ring sets HBM priority: the x
        # pair for group j+1 rides just ahead of adj chunk j, matching the
        # lag-1 PE schedule below so neither stream ever starves the PE
        xw = _N // _XCH
        xts = []
        ats = []

        def load_pair(j):
            xt0 = xpool.tile([_P, xw], bf16, name=f"xt0_{j}", tag=f"xt0_{j}")
            nc.gpsimd.dma_start(xt0[:], xT[0:_P, j * xw:(j + 1) * xw])
            xt1 = xpool.tile([_P, xw], bf16, name=f"xt1_{j}", tag=f"xt1_{j}")
            nc.gpsimd.dma_start(xt1[:], xT[_P:, j * xw:(j + 1) * xw])
            xts.append((xt0, xt1))

        def load_adj(ch):
            at = apool.tile([_P, _CH * _ROWS], f8, name="at", tag="at")
            nc.gpsimd.dma_start(at[:], adjT8[:, ch * _CH * _ROWS:(ch + 1) * _CH * _ROWS])
            ats.append(at)

        load_pair(0)
        load_pair(1)
        for j in range(_NCH - 2):
            load_adj(j)
            load_pair(j + 2)
        load_adj(_NCH - 2)
        load_adj(_NCH - 1)

        # h_aug blocks: [p, kb, f]; col 64 of each block is the ones column
        # (strided memset once, never rewritten)
        h_aug = hpool.tile([_P, _KB, _HA], bf16, name="h_aug", tag="h_aug")
        nc.vector.memset(h_aug[:, :, _OUTF:_HA], 1.0)

        # one PSUM tile per 512-column region so each region's accumulation
        # group closes independently and its epilogue overlaps the rest
        ps_mt = [ps_main.tile([_HA, 512], f32, name=f"ps{mt}", tag=f"ps{mt}")
                 for mt in range(_MT)]

        def h_group(g):
            # h blocks g*8..g*8+7 from x pair g: matmul pairs into one PSUM
            # bank, then one grouped strided copy (alternating engines)
            ph = ps_h.tile([_P, _HG, _OUTF], f32, name="ph", tag="ph")
            xt0, xt1 = xts[g]
            for j in range(_HG):
                lo = j * _P
                nc.tensor.matmul(ph[:, j, :], lhsT=xt0[:, lo:lo + _P],
                                 rhs=w_sb[:, 0:_OUTF], start=True, stop=False)
                nc.tensor.matmul(ph[:, j, :], lhsT=xt1[:, lo:lo + _P],
                                 rhs=w_sb[:, _OUTF:], start=False, stop=True)
            dst = h_aug[:, g * _HG:(g + 1) * _HG, 0:_OUTF]
            if g % 2 == 0:
                nc.scalar.activation(dst, ph[:], ACT.Copy)
            else:
                nc.vector.tensor_copy(dst, ph[:])

        def mm(kb, mt, at, b):
            nc.tensor.matmul(
                ps_mt[mt][:, :],
                lhsT=h_aug[:, kb, :],
                rhs=at[:, b * _ROWS + mt * 512: b * _ROWS + (mt + 1) * 512],
                start=(kb == 0), stop=(kb == _KB - 1),
            )

        # lag-1 interleave: while main chunk j-1 streams through the PE, the
        # scalar/vector copy of h group j lands and adj chunk j arrives
        for g in range(_NCH):
            h_group(g)
            if g >= 1:
                for b in range(_CH):
                    for mt in range(_MT):
                        mm((g - 1) * _CH + b, mt, ats[g - 1], b)

        # last chunk mt-major: each 512-col PSUM region stops early so its
        # epilogue overlaps the remaining matmuls
        out_stage = hpool.tile([_P, (_ROWS // _P) * _OUTF], f32,
                               name="out_stage", tag="out_stage")
        last = ats[_NCH - 1]
        for mt in range(_MT):
            for b in range(_CH):
                mm((_NCH - 1) * _CH + b, mt, last, b)
            for t in range(mt * _TPG, (mt + 1) * _TPG):
                src = ps_mt[mt][:, (t % _TPG) * _P:(t % _TPG + 1) * _P]
                sT = epool.tile([_HA, _P], f32, name="sT", tag="sT")
                if t % 2 == 0:
                    nc.scalar.activation(sT[:], src, ACT.Copy)
                else:
                    nc.vector.tensor_copy(sT[:], src)
                tp = ps_t.tile([_P, _HA], f32, name="tp", tag="tp")
                nc.tensor.transpose(tp[:], sT[:], ident[0:_HA, 0:_HA])
                rec = epool.tile([_P, 1], f32, name="rec", tag="rec")
                nc.vector.reciprocal(rec[:], tp[:, _OUTF:_HA])
                # elu(z) = relu(z) - relu(1 - exp(z)), z = s/deg; the 1/deg
                # division rides along as the ACT per-partition scale
                z = epool.tile([_P, _OUTF], f32, name="z", tag="z")
                nc.scalar.activation(z[:], tp[:, 0:_OUTF], ACT.Relu, scale=rec[:])
                ex = epool.tile([_P, _OUTF], f32, name="ex", tag="ex")
                nc.scalar.activation(ex[:], tp[:, 0:_OUTF], ACT.Exp, scale=rec[:])
                q = epool.tile([_P, _OUTF], f32, name="q", tag="q")
                nc.scalar.activation(q[:], ex[:], ACT.Relu, bias=1.0, scale=-1.0)
                ob = out_stage[:, t * _OUTF:(t + 1) * _OUTF]
                nc.vector.tensor_sub(ob, z[:], q[:])
            s0 = mt * _TPG * _OUTF
            s1 = (mt + 1) * _TPG * _OUTF
            nc.scalar.dma_start(out[:, s0:s1], out_stage[:, s0:s1])

    nc.compile()
    return nc


def _spot_check(out, adj, x, W):
    """Validate a few output rows on host (guards against rare HW transients;
    ~4x the bf16 noise floor). Returns max relative error over the sample."""
    rows = np.arange(_NCORES * 16) * (_N // (_NCORES * 16)) + 7
    h = x.astype(np.float32) @ W.astype(np.float32)
    asel = adj[rows].astype(np.float32)
    s = (asel @ h) / asel.sum(axis=1, keepdims=True)
    want = np.where(s > 0, s, np.expm1(s))
    return np.abs(out[rows] - want).max() / max(np.abs(want).max(), 1e-6)


def kernel(adj, x, W, a=None):
    global _cached_nc, last_results
    from concurrent.futures import ThreadPoolExecutor

    import ml_dtypes
    from concourse.bass_utils import run_bass_kernel_spmd

    adj = np.ascontiguousarray(adj)
    adj8 = adj.astype(ml_dtypes.float8_e4m3)     # 0/1 are exact in fp8
    xT = np.ascontiguousarray(np.asarray(x, dtype=np.float32).T
                              .astype(ml_dtypes.bfloat16))
    Wb = np.asarray(W, dtype=np.float32).astype(ml_dtypes.bfloat16)

    def shard(c):
        # [p, kb*1536+m] = adj8[c*1536+m, kb*128+p]
        s = adj8[c * _ROWS:(c + 1) * _ROWS]      # [1536, 12288]
        return np.ascontiguousarray(
            s.reshape(_ROWS, _KB, _P).transpose(2, 1, 0)
        ).reshape(_P, _KB * _ROWS)

    with ThreadPoolExecutor(_NCORES) as ex:
        shards = list(ex.map(shard, range(_NCORES)))

    if _cached_nc is None:
        _cached_nc = _build_nc()

    in_maps = [{"adjT8": shards[c], "xT": xT, "W": Wb} for c in range(_NCORES)]
    out = None
    for _attempt in range(3):
        try:
            last_results = run_bass_kernel_spmd(
                _cached_nc, in_maps, core_ids=list(range(_NCORES))
            )
        except ModuleNotFoundError:
            # BASS_TRACE set but this image lacks the axon NTFF hook module;
            # rerun with tracing forced off
            import os

            os.environ["BASS_NEVER_TRACE"] = "1"
            last_results = run_bass_kernel_spmd(
                _cached_nc, in_maps, core_ids=list(range(_NCORES))
            )
        out = np.concatenate(
            [
                last_results.results[c]["out"]
                .reshape(_P, _ROWS // _P, _OUTF)
                .transpose(1, 0, 2)
                .reshape(_ROWS, _OUTF)
                for c in range(_NCORES)
            ],
            axis=0,
        ).astype(np.float32)
        if _spot_check(out, adj, x, W) < 1.5e-2:
            break
    return out
